# revision 3
# baseline (speedup 1.0000x reference)
"""CRF energy kernel for Trainium2, SPMD across 8 NeuronCores.

Math: energy = x @ kernel + bias + sm*lb + em*rb, computed as
  outT[u, t] = (w/s).T @ (s*x)T  (+ boundary grid added during evict)
with x quantized to fp8 E3M4 on the host (scale s folded into w).

Schedule (trace-driven): DMA supply (~0.30 MB/us effective) and PE
consumption (0.5 MB chunk / 1.73 us = 0.29 MB/us) are nearly matched,
so the stream must be smooth — 1 MB lumps starved the PE and re-cooled
the HAM clock gate mid-kernel.
  - Per-chunk 512 KB x DMAs in need order on the sync ring; x0 and w
    are k-split so their completion sems (which lag last-byte by the
    16-engine receipt round-trip) gate the first matmuls as early as
    possible.
  - Boundary grid in fp8 E3M4 (it holds ~0.1-scale boundary energies;
    quantization adds <0.5% there) -> 0.5 MB instead of 1 MB bf16,
    placed last on the scalar ring: it gates only the evicts, never
    the matmuls.
  - 24 pre-warm matmuls bridge the global barrier -> x0a landing.
  - Fused DVE evict: ob = psum + bnd (scalar_tensor_tensor), bf16 out,
    paired output DMAs on the scalar ring.
"""

import numpy as np
import ml_dtypes

import concourse.bass as bass
import concourse.mybir as mybir
import concourse.tile as tile
from concourse import bacc
from concourse.bass_utils import run_bass_kernel_spmd
from contextlib import ExitStack

B, T, D, U = 64, 512, 1024, 128
NCORES = 8
MB = B // NCORES            # batches per core
M = MB * T                  # 4096 output rows per core
P = 128
KT = D // P                 # 8 k-tiles
TCH = 512                   # t columns per chunk (= one PSUM bank of f32)
NTC = M // TCH              # 8 chunks per core
NWARM = 24                  # HAM pre-warm dummy matmuls

BF16 = mybir.dt.bfloat16
F32 = mybir.dt.float32
FP8 = mybir.dt.float8e3

_CACHE = {}
LAST_RESULTS = None


def build_nc():
    nc = bacc.Bacc(target_bir_lowering=False)
    xq = nc.declare_dram_parameter("xq", [NTC * P, KT * TCH], FP8, isOutput=False)
    wt = nc.declare_dram_parameter("wt", [P, KT * U], BF16, isOutput=False)
    bnd = nc.declare_dram_parameter("bnd", [U, NTC * TCH], FP8, isOutput=False)
    out = nc.declare_dram_parameter("out", [NTC * U, TCH], BF16, isOutput=True)

    with ExitStack() as ctx:
        tc = ctx.enter_context(tile.TileContext(nc))
        consts = ctx.enter_context(tc.tile_pool(name="consts", bufs=1))
        xpool = ctx.enter_context(tc.tile_pool(name="xpool", bufs=8))
        opool = ctx.enter_context(tc.tile_pool(name="opool", bufs=2))
        pps = ctx.enter_context(tc.tile_pool(name="pps", bufs=4, space="PSUM"))
        pwrm = ctx.enter_context(tc.tile_pool(name="pwrm", bufs=4, space="PSUM"))

        # PE pre-warm bridge: barrier -> x0a landing.
        wrm = consts.tile([P, P], BF16)
        nc.vector.memset(wrm, 0.0)
        for i in range(NWARM):
            pw = pwrm.tile([P, P], F32, tag="pw", name="pw")
            nc.tensor.matmul(pw, lhsT=wrm, rhs=wrm, start=True, stop=True)

        w_sb = consts.tile([P, KT, U], BF16)
        bnd_sb = consts.tile([U, NTC, TCH], FP8)
        xa = [xpool.tile([P, KT, TCH], FP8, tag="xa", name=f"xa{c}")
              for c in range(NTC)]
        H = KT // 2
        # sync ring: the x stream in need order, x0 k-split for an early
        # first-matmul gate
        nc.sync.dma_start(out=xa[0][:, 0:H, :], in_=xq[0:P, 0:H * TCH])
        nc.sync.dma_start(out=xa[0][:, H:, :], in_=xq[0:P, H * TCH:])
        for c in range(1, 5):
            nc.sync.dma_start(out=xa[c], in_=xq[c * P:(c + 1) * P, :])
        for c in range(5, NTC):
            # k-split the tail chunks: their completion sems lag last-byte
            # by the straggler-engine backlog (~4 us at ring end); halves
            # let the warm PE start each chunk ~0.6 us earlier
            nc.sync.dma_start(out=xa[c][:, 0:H, :],
                              in_=xq[c * P:(c + 1) * P, 0:H * TCH])
            nc.sync.dma_start(out=xa[c][:, H:, :],
                              in_=xq[c * P:(c + 1) * P, H * TCH:])
        # scalar ring: w (k-split), boundary grid, then paired outputs
        nc.scalar.dma_start(out=w_sb[:, 0:H, :], in_=wt[:, 0:H * U])
        nc.scalar.dma_start(out=w_sb[:, H:, :], in_=wt[:, H * U:])
        nc.scalar.dma_start(out=bnd_sb[:, 0:NTC // 2, :], in_=bnd[:, 0:NTC // 2 * TCH])
        nc.scalar.dma_start(out=bnd_sb[:, NTC // 2:, :], in_=bnd[:, NTC // 2 * TCH:])

        ob = None
        for c in range(NTC):
            ps = pps.tile([U, TCH], F32, tag="ps", name="ps")
            for k in range(KT):
                nc.tensor.matmul(ps, lhsT=w_sb[:, k, :], rhs=xa[c][:, k, :],
                                 start=(k == 0), stop=(k == KT - 1))
            if c % 2 == 0:
                ob = opool.tile([U, 2, TCH], BF16, tag="ob", name="ob")
            # fused evict: ob = bnd/16 + psum (DVE, fp8+f32 -> bf16; the
            # grid is host-scaled by 16 to clear E3M4's subnormal range)
            nc.vector.scalar_tensor_tensor(
                out=ob[:, c % 2, :], in0=bnd_sb[:, c, :], scalar=1.0 / 16.0,
                in1=ps, op0=mybir.AluOpType.mult, op1=mybir.AluOpType.add)
            if c % 2 == 1:
                dst = out[(c - 1) * U:(c + 1) * U, :]
                nc.scalar.dma_start(
                    out=dst.rearrange("(two u) t -> u two t", two=2), in_=ob)
    nc.finalize()
    return nc


def _shift_right(m):
    z = np.zeros_like(m[:, :1])
    return np.concatenate([z, m[:, :-1]], axis=1)


def _shift_left(m):
    z = np.zeros_like(m[:, :1])
    return np.concatenate([m[:, 1:], z], axis=1)


def kernel(x, mask, kernel, bias, left_boundary, right_boundary):
    global LAST_RESULTS
    x = np.asarray(x, dtype=np.float32)
    assert x.shape == (B, T, D), x.shape
    mask = np.asarray(mask)
    kern = np.asarray(kernel, dtype=np.float32)
    bias = np.asarray(bias, dtype=np.float32)
    lb = np.asarray(left_boundary, dtype=np.float32)
    rb = np.asarray(right_boundary, dtype=np.float32)

    if "nc" not in _CACHE:
        _CACHE["nc"] = build_nc()
    nc = _CACHE["nc"]

    bf = ml_dtypes.bfloat16
    e3 = ml_dtypes.float8_e3m4

    amax = float(np.abs(x).max())
    s = min(4.0, 14.0 / max(amax, 1e-6))
    xs = (x.reshape(B * T, D) * np.float32(s)).astype(e3)     # [B*T, D] fp8
    wt_b = (kern * np.float32(1.0 / s)).astype(bf)            # [D, U]
    wt_l = np.ascontiguousarray(
        wt_b.reshape(KT, P, U).transpose(1, 0, 2)).reshape(P, KT * U)

    m = mask.astype(np.float32)                               # [B, T]
    sm = (m > _shift_right(m)).astype(np.float32)
    em = (_shift_left(m) > m).astype(np.float32)

    in_maps = []
    for c in range(NCORES):
        xc = xs[c * M:(c + 1) * M]                            # [M, D] fp8
        xc_l = np.ascontiguousarray(
            xc.reshape(NTC, TCH, KT, P).transpose(0, 3, 2, 1)
        ).reshape(NTC * P, KT * TCH)
        sm_c = sm[c * MB:(c + 1) * MB].reshape(M)
        em_c = em[c * MB:(c + 1) * MB].reshape(M)
        bnd_c = (sm_c[:, None] * lb[None, :] + em_c[:, None] * rb[None, :]
                 + bias[None, :]).T * np.float32(16.0)        # [U, M], x16
        in_maps.append({"xq": xc_l, "wt": wt_l,
                        "bnd": np.ascontiguousarray(bnd_c).astype(e3)})

    res = run_bass_kernel_spmd(nc, in_maps, core_ids=list(range(NCORES)))
    LAST_RESULTS = res
    outs = []
    for c in range(NCORES):
        o = np.asarray(res.results[c]["out"]).astype(np.float32)  # [NTC*U, TCH]
        outs.append(o.reshape(NTC, U, TCH).transpose(0, 2, 1).reshape(M, U))
    return np.concatenate(outs, axis=0).reshape(B, T, U)
